# revision 29
# baseline (speedup 1.0000x reference)
"""Trainium2 Bass kernel: causal multi-head self-attention block (B=8, T=1024, E=768, H=12).

Sharding: data-parallel over batch - one batch element per NeuronCore, 8 cores,
no collectives. Each core computes the full attention block for its batch row.

(v7-v9 experiments regressed or were noise-level: outproj_a-as-filler,
2-kc DMA waves, alternating eviction paths, halves=1 for norm(5,1);
all reverted to the v6 configuration.)

v6 (over v3): input DMAs batched into wide strided descriptors (the Sync
queue pays ~620ns per dma_start issue - 18 narrow wave-1 DMAs alone cost
~11us of issue time); the last pair's av_norm(5,0) is emitted before the
kt4-7 exps so outproj_b(0..3) unblocks during outproj_a instead of 2.4us
after it; deeper opool. (v4/v5 experiments that regressed are reverted:
5+1 outproj split, keep_z, single-hop DVE merge, scores-before-av_mm.)

v3 (over v2): softmax denominator materialized on 64 partitions by widening the
V stationary with a 64-column ones block (the AV matmul broadcasts the
denominator for free), reciprocal via partition-crossed Ln/Exp on ScalarE - no
GpSimd partition_broadcast anywhere in the pair loop; output projection tail
merges the early-pair partial with DVE adds instead of identity matmuls
(-6144 PE columns), runs on 3 rotating PSUM slots from the att pool, and
writes bf16 output with one DMA per row-tile.

Self-contained: hardcodes all shapes; only imports concourse (installed
system-wide) and ml_dtypes.
"""

import numpy as np
import ml_dtypes

B, T, E, H, Dh = 8, 1024, 768, 12, 64
F3 = 3 * E            # 2304
KC = E // 128         # 6 e-chunks
MT = T // 128         # 8 t-tiles
NPAIR = H // 2        # 6 head pairs
SCALE = 1.0 / float(np.sqrt(Dh))

_NC_CACHE = None


def build_nc():
    import concourse.mybir as mybir
    from concourse import bacc
    from concourse.tile import TileContext

    bf = mybir.dt.bfloat16
    f32 = mybir.dt.float32
    EXP = mybir.ActivationFunctionType.Exp
    LN = mybir.ActivationFunctionType.Ln
    COPY = mybir.ActivationFunctionType.Copy
    ACT_SET_LN_EXP = 6  # natural_log_exp_and_others: holds both Ln and Exp

    nc = bacc.Bacc("TRN2", target_bir_lowering=False, debug=False, num_devices=B, name="attn_dp3")

    XT_ext = nc.declare_dram_parameter("XT", [E, T], bf, isOutput=False)
    W1_ext = nc.declare_dram_parameter("W1", [E, F3], bf, isOutput=False)
    b1_ext = nc.declare_dram_parameter("b1", [F3], f32, isOutput=False)
    b1q_ext = nc.declare_dram_parameter("b1q", [128, 6], f32, isOutput=False)
    W2_ext = nc.declare_dram_parameter("W2", [E, E], bf, isOutput=False)
    b2_ext = nc.declare_dram_parameter("b2", [E], f32, isOutput=False)
    out_ext = nc.declare_dram_parameter("out", [T, E], bf, isOutput=True)

    with TileContext(nc) as tc:
        with (
            tc.tile_pool(name="persist", bufs=1) as persist,
            tc.tile_pool(name="stage", bufs=1) as stage,
            tc.tile_pool(name="ptpool", bufs=13) as ptpool,
            tc.tile_pool(name="zsbpool", bufs=3) as zsbpool,
            tc.tile_pool(name="dlnpool", bufs=1) as dlnpool,
            tc.tile_pool(name="recpool", bufs=2) as recpool,
            tc.tile_pool(name="opool", bufs=3) as opool,
            tc.tile_pool(name="att", bufs=3, space="PSUM") as att,
            tc.tile_pool(name="acc", bufs=2, space="PSUM") as acc,
        ):
            # ---- input DMAs first: the whole kernel is gated on W1/XT landing.
            # Batched strided descriptors: the Sync queue spends ~620ns ISSUING
            # each dma_start, so wave 1 uses 7 wide DMAs (3 kc-chunks apiece)
            # instead of 18 narrow ones - the old issue stream alone took ~11us.
            XT = persist.tile([128, KC, T], bf, tag="XT")
            W1bf = persist.tile([128, KC, F3], bf, tag="W1bf")
            W2bf = persist.tile([128, KC, E], bf, tag="W2bf")

            def dram3(ext, r0, r1, c0, c1, axis="t"):
                return ext[r0 * 128:r1 * 128, c0:c1].rearrange(
                    f"(kc p) {axis} -> p kc {axis}", p=128)

            # W1 arrives host-reordered as m-block pairs [m0|m6|m1|m7|...|m5|m11|V]
            for g in range(2):
                a, b = 3 * g, 3 * (g + 1)
                nc.sync.dma_start(out=XT[:, a:b, 0:512], in_=dram3(XT_ext, a, b, 0, 512))
                nc.sync.dma_start(out=W1bf[:, a:b, 0:256], in_=dram3(W1_ext, a, b, 0, 256, "c"))
            # Q-bias lands right behind wave1: it gates the first QK evictions
            b1qk = persist.tile([128, 6], f32, tag="b1qk")
            nc.sync.dma_start(out=b1qk[:], in_=b1q_ext[:, :])
            nc.sync.dma_start(out=W1bf[:, :, 256:512], in_=dram3(W1_ext, 0, KC, 256, 512, "c"))
            for g in range(2):
                a, b = 3 * g, 3 * (g + 1)
                nc.sync.dma_start(out=XT[:, a:b, 512:1024], in_=dram3(XT_ext, a, b, 512, 1024))
            # V columns (gate vproj) + V bias
            nc.sync.dma_start(out=W1bf[:, :, 1536:2304], in_=dram3(W1_ext, 0, KC, 1536, 2304, "c"))
            b1v_f = stage.tile([1, E], f32, tag="rowstage")
            nc.sync.dma_start(out=b1v_f[:], in_=b1_ext[None, 1536:2304])
            b1vb = persist.tile([128, E], f32, tag="b1vb")
            nc.gpsimd.partition_broadcast(b1vb[:], b1v_f[:])
            # remaining QK weight columns (pairs 2..5)
            nc.sync.dma_start(out=W1bf[:, :, 512:1536], in_=dram3(W1_ext, 0, KC, 512, 1536, "c"))

            # per-partition bias (host-prepared [128,6] layout) for the Q part of b1
            # (K bias dropped: softmax-invariant per-query shift.)
            b2_f = stage.tile([1, E], f32, tag="rowstage")
            nc.sync.dma_start(out=b2_f[:], in_=b2_ext[None, :])
            b2b = persist.tile([128, E], f32, tag="b2b")
            nc.gpsimd.partition_broadcast(b2b[:], b2_f[:])

            def w2_dma():
                nc.sync.dma_start(out=W2bf[:, :, :], in_=dram3(W2_ext, 0, KC, 0, E, "c"))

            # One activation-table load for the whole kernel (covers Exp + Ln).
            nc.scalar.add_instruction(mybir.InstLoadActFuncSet(
                name=nc.get_next_instruction_name(), ins=[], outs=[],
                act_func_set_id=ACT_SET_LN_EXP))

            # ---- constants ----
            # multiplicative causal mask for the diagonal 128x128 block, for
            # both heads of a pair: mask[k, h01, q] = 1 where q >= k else 0
            diagmask = persist.tile([128, 2, 128], bf, tag="diagmask")
            nc.gpsimd.memset(diagmask[:], 1.0)
            for h01 in range(2):
                nc.gpsimd.affine_select(
                    out=diagmask[:, h01, :], in_=diagmask[:, h01, :],
                    compare_op=mybir.AluOpType.is_ge, fill=0.0, base=0,
                    pattern=[[1, 128]], channel_multiplier=-1,
                )

            # QK[p, m, t]: m 0..5 = Q^T blocks (f rows m*128..), m 6..11 = K^T blocks
            QK = persist.tile([128, 12, T], bf, tag="QK")

            # W1 column slot for logical m-block under the host reorder
            # [m0, m6, m1, m7, m2, m8, m3, m9, m4, m10, m5, m11]
            QKSLOT = {}
            for i in range(6):
                QKSLOT[i] = 2 * i
                QKSLOT[6 + i] = 2 * i + 1

            def qk_mtile(m):
                # two 512-column chains, kc-interleaved so consecutive PE
                # writes alternate PSUM banks
                s = QKSLOT[m]
                psA = acc.tile([128, 512], f32, tag="acc")
                psB = acc.tile([128, 512], f32, tag="acc")
                for kc in range(KC):
                    nc.tensor.matmul(
                        psA[:], W1bf[:, kc, s * 128:(s + 1) * 128],
                        XT[:, kc, 0:512], start=(kc == 0), stop=(kc == KC - 1))
                    nc.tensor.matmul(
                        psB[:], W1bf[:, kc, s * 128:(s + 1) * 128],
                        XT[:, kc, 512:1024], start=(kc == 0), stop=(kc == KC - 1))
                # evictions on DVE: ScalarE is the critical engine (exp) and
                # must not queue eviction work ahead of the exp stream
                if m < 6:
                    nc.vector.tensor_scalar_add(QK[:, m, 0:512], psA[:], b1qk[:, m:m + 1])
                    nc.vector.tensor_scalar_add(QK[:, m, 512:1024], psB[:], b1qk[:, m:m + 1])
                else:
                    nc.vector.tensor_copy(QK[:, m, 0:512], psA[:])
                    nc.vector.tensor_copy(QK[:, m, 512:1024], psB[:])

            # ---- V projection into Vg[t-part, kt, h, 0:64]; cols 64:128 hold a
            # ones block so the AV matmul lands the softmax denominator,
            # replicated across PSUM partitions 64:128, for free.
            Vg = persist.tile([128, MT, H, 128], bf, tag="Vg")
            for mt in range(MT):
                nc.gpsimd.memset(Vg[:, mt, :, Dh:128], 1.0)

            def vproj(mts):
                for mt in mts:
                    psA = acc.tile([128, 512], f32, tag="acc")
                    psB = acc.tile([128, 512], f32, tag="acc")
                    for kc in range(KC):
                        nc.tensor.matmul(
                            psA[:, 0:512], XT[:, kc, mt * 128:(mt + 1) * 128],
                            W1bf[:, kc, 1536:2048],
                            start=(kc == 0), stop=(kc == KC - 1))
                        nc.tensor.matmul(
                            psB[:, 0:256], XT[:, kc, mt * 128:(mt + 1) * 128],
                            W1bf[:, kc, 2048:2304],
                            start=(kc == 0), stop=(kc == KC - 1))
                    nc.vector.tensor_add(
                        Vg[:, mt, 0:8, 0:Dh],
                        psA[:].rearrange("p (h d) -> p h d", d=Dh),
                        b1vb[:, 0:512].rearrange("p (h d) -> p h d", d=Dh))
                    nc.vector.tensor_add(
                        Vg[:, mt, 8:12, 0:Dh],
                        psB[:, 0:256].rearrange("p (h d) -> p h d", d=Dh),
                        b1vb[:, 512:768].rearrange("p (h d) -> p h d", d=Dh))

            # ---- attention ----
            ZT = persist.tile([128, NPAIR, T], bf, tag="ZT")

            def scores_kt(hp, kt, pts):
                L = T - kt * 128
                ptile = ptpool.tile([128, 2, 1024], bf, tag="pt", bufs=10)
                for c_off in range(0, L, 512):
                    n = min(512, L - c_off)
                    sc = att.tile([128, 2, 512], f32, tag="att")
                    for h01 in range(2):
                        base = h01 * 64
                        nc.tensor.matmul(
                            sc[:, h01, 0:n],
                            QK[base:base + 64, 6 + hp, kt * 128:(kt + 1) * 128],
                            QK[base:base + 64, hp, kt * 128 + c_off:kt * 128 + c_off + n],
                            start=True, stop=True)
                    nc.scalar.activation(
                        ptile[:, :, c_off:c_off + n], sc[:, :, 0:n], EXP, scale=SCALE)
                # causal mask on the diagonal 128x128 block, both heads in one op
                nc.vector.tensor_mul(ptile[:, :, 0:128], ptile[:, :, 0:128], diagmask[:])
                pts[kt] = (ptile, 0)

            def scores_kt67(hp, pts):
                # kt=6 (256 cols) and kt=7 (128 cols) share one PSUM tile
                # ([128,2,384] f32 = 3KB/part, fits the 2-bank att slot) and a
                # single exp - one ScalarE instruction instead of two.
                ptile = ptpool.tile([128, 2, 384], bf, tag="pt67", bufs=3)
                # full [2,512] slot so h01=1 starts on the PSUM bank boundary
                # (matmul outputs must not straddle banks); only 0:384 used
                sc = att.tile([128, 2, 512], f32, tag="att")
                for j, kt in enumerate((6, 7)):
                    off = 0 if j == 0 else 256
                    L = T - kt * 128
                    for h01 in range(2):
                        base = h01 * 64
                        nc.tensor.matmul(
                            sc[:, h01, off:off + L],
                            QK[base:base + 64, 6 + hp, kt * 128:(kt + 1) * 128],
                            QK[base:base + 64, hp, kt * 128:T],
                            start=True, stop=True, skip_group_check=True)
                nc.scalar.activation(ptile[:], sc[:, :, 0:384], EXP, scale=SCALE)
                nc.vector.tensor_mul(ptile[:, :, 0:128], ptile[:, :, 0:128], diagmask[:])
                nc.vector.tensor_mul(ptile[:, :, 256:384], ptile[:, :, 256:384], diagmask[:])
                pts[6] = (ptile, 0)
                pts[7] = (ptile, 256)

            def av_mm(hp, c, pts, keep_z=False):
                z = att.tile([128, 2, 512], f32, tag="att")
                kts = list(range(0, min(MT, 4 * (c + 1))))
                for kt in kts:
                    zoff = max(kt * 128 - c * 512, 0)
                    n = 512 - zoff
                    poff = max(c * 512 - kt * 128, 0)
                    ptile, pbase = pts[kt]
                    for h01 in range(2):
                        nc.tensor.matmul(
                            z[:, h01, zoff:zoff + n],
                            Vg[:, kt, 2 * hp + h01, :],
                            ptile[:, h01, pbase + poff:pbase + poff + n],
                            start=(kt == kts[0]), stop=(kt == kts[-1]),
                            skip_group_check=True)
                if keep_z:
                    # last pair: normalize straight from PSUM (saves the copy
                    # latency on the critical tail); z slot held until the muls.
                    return z
                # single bf16 copy (Z rows + denominator rows) frees the PSUM
                # slot; normalization is emitted later (av_norm) so ScalarE
                # keeps prioritizing the exp stream.
                zsb = zsbpool.tile([128, 2, 512], bf, tag="zsb")
                nc.vector.tensor_copy(zsb[:], z[:])
                return zsb

            def av_norm(hp, c, zsb, halves=1):
                # reciprocal of the denominator via Ln -> Exp(-x); the Ln reads
                # the denominator rows (partitions 64:128) and writes partitions
                # 0:64 (partition-crossed ACT), so the muls are lane-aligned.
                # halves=2 pipelines the chain in 256-col pieces for the
                # latency-critical last pair. zsb may be the live PSUM z tile
                # (keep_z path): the muls then read Z straight from PSUM.
                w = 512 // halves
                for hf in range(halves):
                    lo, hi = hf * w, (hf + 1) * w
                    dln = dlnpool.tile([64, 2, 512], f32, tag="dln")
                    nc.scalar.activation(dln[:, :, 0:w], zsb[Dh:128, :, lo:hi], LN)
                    rec = recpool.tile([64, 2, 512], bf, tag="rec")
                    nc.scalar.activation(rec[:, :, 0:w], dln[:, :, 0:w], EXP, scale=-1.0)
                    for h01 in range(2):
                        nc.vector.tensor_mul(
                            ZT[h01 * 64:(h01 + 1) * 64, hp, c * 512 + lo:c * 512 + hi],
                            zsb[0:Dh, h01, lo:hi], rec[:, h01, 0:w])

            # ---- output projection, split-K: pairs 0-3 accumulated early into
            # outA, pairs 4-5 at the tail; the merge is a ScalarE cast + DVE add
            # (no identity matmuls on the PE).
            outA = persist.tile([128, MT, E], bf, tag="outA")

            def outproj_a(mts):
                for mt in mts:
                    psA = acc.tile([128, 512], f32, tag="acc")
                    psB = acc.tile([128, 512], f32, tag="acc")
                    for pc in range(4):
                        nc.tensor.matmul(
                            psA[:], ZT[:, pc, mt * 128:(mt + 1) * 128],
                            W2bf[:, pc, 0:512], start=(pc == 0), stop=(pc == 3))
                        nc.tensor.matmul(
                            psB[:, 0:256], ZT[:, pc, mt * 128:(mt + 1) * 128],
                            W2bf[:, pc, 512:768], start=(pc == 0), stop=(pc == 3))
                    nc.vector.tensor_add(outA[:, mt, 0:512], psA[:], b2b[:, 0:512])
                    nc.vector.tensor_add(outA[:, mt, 512:768], psB[:, 0:256], b2b[:, 512:768])

            def outproj_b(mts):
                # pairs 4..5 tail on att-pool PSUM slots (3 in flight); the outA
                # partial (which already carries b2) is merged during eviction:
                # ScalarE casts the PSUM chain to bf16, DVE adds outA, one DMA.
                for mt in mts:
                    ps = att.tile([128, E], f32, tag="att")
                    for pc in range(4, KC):
                        nc.tensor.matmul(
                            ps[:, 0:512], ZT[:, pc, mt * 128:(mt + 1) * 128],
                            W2bf[:, pc, 0:512], start=(pc == 4), stop=(pc == KC - 1))
                        nc.tensor.matmul(
                            ps[:, 512:768], ZT[:, pc, mt * 128:(mt + 1) * 128],
                            W2bf[:, pc, 512:768], start=(pc == 4), stop=(pc == KC - 1),
                            skip_group_check=True)
                    osb = opool.tile([128, E], bf, tag="osb")
                    if mt == MT - 1:
                        # last tile: one-hop mixed-dtype add, shorter final drain
                        nc.vector.tensor_add(osb[:], ps[:], outA[:, mt, :])
                    else:
                        tmpb = opool.tile([128, E], bf, tag="tmpb")
                        nc.scalar.activation(tmpb[:], ps[:], COPY)
                        nc.vector.tensor_add(osb[:], tmpb[:], outA[:, mt, :])
                    nc.sync.dma_start(
                        out=out_ext[mt * 128:(mt + 1) * 128, :], in_=osb[:])

            # Software-pipelined pair loop: pair hp+1's first score batch is
            # emitted inside pair hp's body (before av_norm(hp,1)) so the next
            # exps queue on ScalarE ahead of the non-critical normalization
            # work instead of behind it. qk tiles are computed two pairs ahead.
            qk_mtile(0)
            qk_mtile(6)

            pts_cur = {}
            for kt in range(4):
                scores_kt(0, kt, pts_cur)
            qk_mtile(1)
            qk_mtile(7)

            for hp in range(NPAIR):
                if hp == 0:
                    vproj(range(MT))
                zsb0 = av_mm(hp, 0, pts_cur)
                if hp == NPAIR - 1:
                    # last pair: norm(5,0) jumps the ScalarE queue ahead of the
                    # kt4-7 exps - it only needs ptiles 0-3, and outproj_b(0..3)
                    # is gated on its ZT writes.
                    av_norm(hp, 0, zsb0, halves=2)
                scores_kt(hp, 4, pts_cur)
                scores_kt(hp, 5, pts_cur)
                scores_kt67(hp, pts_cur)
                if hp != NPAIR - 1:
                    av_norm(hp, 0, zsb0)
                if hp + 2 < NPAIR:
                    qk_mtile(hp + 2)
                    qk_mtile(6 + hp + 2)
                zsb1 = av_mm(hp, 1, pts_cur)
                if hp == 1:
                    w2_dma()
                pts_next = {}
                if hp + 1 < NPAIR:
                    for kt in range(4):
                        scores_kt(hp + 1, kt, pts_next)
                av_norm(hp, 1, zsb1, halves=(2 if hp == NPAIR - 1 else 1))
                if hp == 4:
                    outproj_a(range(0, 4))
                pts_cur = pts_next

            outproj_a(range(4, MT))
            outproj_b(range(0, 4))
            outproj_b(range(4, MT))

    nc.compile()
    return nc


def _get_nc():
    global _NC_CACHE
    if _NC_CACHE is None:
        _NC_CACHE = build_nc()
    return _NC_CACHE


def _in_maps(X, W1, b1, W2, b2):
    bfdt = ml_dtypes.bfloat16
    X = np.asarray(X, dtype=np.float32)
    assert X.shape == (B, T, E)
    W1 = np.asarray(W1, dtype=np.float32)
    # reorder QK columns into m-block pairs [m0|m6|m1|m7|...|m5|m11], V last,
    # matching the device-side QKSLOT map and DMA wave order
    order = []
    for i in range(6):
        order += [i, 6 + i]
    W1r = np.concatenate(
        [W1[:, m * 128:(m + 1) * 128] for m in order] + [W1[:, 1536:2304]], axis=1)
    W1b = np.ascontiguousarray(W1r.astype(bfdt))
    W2b = np.ascontiguousarray(np.asarray(W2, dtype=np.float32).astype(bfdt))
    b1 = np.ascontiguousarray(np.asarray(b1, dtype=np.float32))
    b1q = np.ascontiguousarray(b1[0:768].reshape(6, 128).T)
    b2 = np.ascontiguousarray(np.asarray(b2, dtype=np.float32))
    XTs = [np.ascontiguousarray(X[i].T.astype(bfdt)) for i in range(B)]
    return [
        {"XT": XTs[i], "W1": W1b, "b1": b1, "b1q": b1q, "W2": W2b, "b2": b2}
        for i in range(B)
    ]


def kernel(X, W1, b1, W2, b2):
    from concourse.bass_utils import run_bass_kernel_spmd

    nc = _get_nc()
    res = run_bass_kernel_spmd(nc, _in_maps(X, W1, b1, W2, b2), core_ids=list(range(B)))
    return np.stack([np.asarray(res.results[i]["out"], dtype=np.float32) for i in range(B)])


def kernel_traced(X, W1, b1, W2, b2, tmpdir=None):
    """Like kernel() but with neuron-profile tracing; returns (out, BassKernelResults)."""
    from concourse.bass_utils import run_bass_kernel_spmd

    nc = _get_nc()
    res = run_bass_kernel_spmd(
        nc, _in_maps(X, W1, b1, W2, b2), core_ids=list(range(B)),
        trace=True, tmpdir=tmpdir,
    )
    out = np.stack([np.asarray(res.results[i]["out"], dtype=np.float32) for i in range(B)])
    return out, res


# revision 30
# speedup vs baseline: 1.0002x; 1.0002x over previous
"""Trainium2 Bass kernel: causal multi-head self-attention block (B=8, T=1024, E=768, H=12).

Sharding: data-parallel over batch - one batch element per NeuronCore, 8 cores,
no collectives. Each core computes the full attention block for its batch row.

(v7-v9 experiments regressed or were noise-level: outproj_a-as-filler,
2-kc DMA waves, alternating eviction paths, halves=1 for norm(5,1);
all reverted to the v6 configuration.)

v6 (over v3): input DMAs batched into wide strided descriptors (the Sync
queue pays ~620ns per dma_start issue - 18 narrow wave-1 DMAs alone cost
~11us of issue time); the last pair's av_norm(5,0) is emitted before the
kt4-7 exps so outproj_b(0..3) unblocks during outproj_a instead of 2.4us
after it; deeper opool. (v4/v5 experiments that regressed are reverted:
5+1 outproj split, keep_z, single-hop DVE merge, scores-before-av_mm.)

v3 (over v2): softmax denominator materialized on 64 partitions by widening the
V stationary with a 64-column ones block (the AV matmul broadcasts the
denominator for free), reciprocal via partition-crossed Ln/Exp on ScalarE - no
GpSimd partition_broadcast anywhere in the pair loop; output projection tail
merges the early-pair partial with DVE adds instead of identity matmuls
(-6144 PE columns), runs on 3 rotating PSUM slots from the att pool, and
writes bf16 output with one DMA per row-tile.

Self-contained: hardcodes all shapes; only imports concourse (installed
system-wide) and ml_dtypes.
"""

import numpy as np
import ml_dtypes

B, T, E, H, Dh = 8, 1024, 768, 12, 64
F3 = 3 * E            # 2304
KC = E // 128         # 6 e-chunks
MT = T // 128         # 8 t-tiles
NPAIR = H // 2        # 6 head pairs
SCALE = 1.0 / float(np.sqrt(Dh))

_NC_CACHE = None


def build_nc():
    import concourse.mybir as mybir
    from concourse import bacc
    from concourse.tile import TileContext

    bf = mybir.dt.bfloat16
    f32 = mybir.dt.float32
    EXP = mybir.ActivationFunctionType.Exp
    LN = mybir.ActivationFunctionType.Ln
    COPY = mybir.ActivationFunctionType.Copy
    ACT_SET_LN_EXP = 6  # natural_log_exp_and_others: holds both Ln and Exp

    nc = bacc.Bacc("TRN2", target_bir_lowering=False, debug=False, num_devices=B, name="attn_dp3")

    XT_ext = nc.declare_dram_parameter("XT", [E, T], bf, isOutput=False)
    W1_ext = nc.declare_dram_parameter("W1", [E, F3], bf, isOutput=False)
    b1_ext = nc.declare_dram_parameter("b1", [F3], f32, isOutput=False)
    b1q_ext = nc.declare_dram_parameter("b1q", [128, 6], f32, isOutput=False)
    W2_ext = nc.declare_dram_parameter("W2", [E, E], bf, isOutput=False)
    b2_ext = nc.declare_dram_parameter("b2", [E], f32, isOutput=False)
    out_ext = nc.declare_dram_parameter("out", [T, E], bf, isOutput=True)

    with TileContext(nc) as tc:
        with (
            tc.tile_pool(name="persist", bufs=1) as persist,
            tc.tile_pool(name="stage", bufs=1) as stage,
            tc.tile_pool(name="ptpool", bufs=13) as ptpool,
            tc.tile_pool(name="zsbpool", bufs=3) as zsbpool,
            tc.tile_pool(name="dlnpool", bufs=1) as dlnpool,
            tc.tile_pool(name="recpool", bufs=2) as recpool,
            tc.tile_pool(name="opool", bufs=3) as opool,
            tc.tile_pool(name="att", bufs=3, space="PSUM") as att,
            tc.tile_pool(name="acc", bufs=2, space="PSUM") as acc,
        ):
            # ---- input DMAs first: the whole kernel is gated on W1/XT landing.
            # Batched strided descriptors: the Sync queue spends ~620ns ISSUING
            # each dma_start, so wave 1 uses 7 wide DMAs (3 kc-chunks apiece)
            # instead of 18 narrow ones - the old issue stream alone took ~11us.
            XT = persist.tile([128, KC, T], bf, tag="XT")
            W1bf = persist.tile([128, KC, F3], bf, tag="W1bf")
            W2bf = persist.tile([128, KC, E], bf, tag="W2bf")

            def dram3(ext, r0, r1, c0, c1, axis="t"):
                return ext[r0 * 128:r1 * 128, c0:c1].rearrange(
                    f"(kc p) {axis} -> p kc {axis}", p=128)

            # W1 arrives host-reordered as m-block pairs [m0|m6|m1|m7|...|m5|m11|V]
            for g in range(2):
                a, b = 3 * g, 3 * (g + 1)
                nc.sync.dma_start(out=XT[:, a:b, 0:512], in_=dram3(XT_ext, a, b, 0, 512))
                nc.sync.dma_start(out=W1bf[:, a:b, 0:256], in_=dram3(W1_ext, a, b, 0, 256, "c"))
            # Q-bias lands right behind wave1: it gates the first QK evictions
            b1qk = persist.tile([128, 6], f32, tag="b1qk")
            nc.sync.dma_start(out=b1qk[:], in_=b1q_ext[:, :])
            nc.sync.dma_start(out=W1bf[:, :, 256:512], in_=dram3(W1_ext, 0, KC, 256, 512, "c"))
            for g in range(2):
                a, b = 3 * g, 3 * (g + 1)
                nc.sync.dma_start(out=XT[:, a:b, 512:1024], in_=dram3(XT_ext, a, b, 512, 1024))
            # V columns (gate vproj) + V bias
            nc.sync.dma_start(out=W1bf[:, :, 1536:2304], in_=dram3(W1_ext, 0, KC, 1536, 2304, "c"))
            b1v_f = stage.tile([1, E], f32, tag="rowstage")
            nc.sync.dma_start(out=b1v_f[:], in_=b1_ext[None, 1536:2304])
            b1vb = persist.tile([128, E], f32, tag="b1vb")
            nc.gpsimd.partition_broadcast(b1vb[:], b1v_f[:])
            # remaining QK weight columns (pairs 2..5)
            nc.sync.dma_start(out=W1bf[:, :, 512:1536], in_=dram3(W1_ext, 0, KC, 512, 1536, "c"))

            # per-partition bias (host-prepared [128,6] layout) for the Q part of b1
            # (K bias dropped: softmax-invariant per-query shift.)
            b2_f = stage.tile([1, E], f32, tag="rowstage")
            nc.sync.dma_start(out=b2_f[:], in_=b2_ext[None, :])
            b2b = persist.tile([128, E], f32, tag="b2b")
            nc.gpsimd.partition_broadcast(b2b[:], b2_f[:])

            def w2_dma():
                nc.sync.dma_start(out=W2bf[:, :, :], in_=dram3(W2_ext, 0, KC, 0, E, "c"))

            # One activation-table load for the whole kernel (covers Exp + Ln).
            nc.scalar.add_instruction(mybir.InstLoadActFuncSet(
                name=nc.get_next_instruction_name(), ins=[], outs=[],
                act_func_set_id=ACT_SET_LN_EXP))

            # ---- constants ----
            # multiplicative causal mask for the diagonal 128x128 block, for
            # both heads of a pair: mask[k, h01, q] = 1 where q >= k else 0
            diagmask = persist.tile([128, 2, 128], bf, tag="diagmask")
            nc.gpsimd.memset(diagmask[:], 1.0)
            for h01 in range(2):
                nc.gpsimd.affine_select(
                    out=diagmask[:, h01, :], in_=diagmask[:, h01, :],
                    compare_op=mybir.AluOpType.is_ge, fill=0.0, base=0,
                    pattern=[[1, 128]], channel_multiplier=-1,
                )

            # QK[p, m, t]: m 0..5 = Q^T blocks (f rows m*128..), m 6..11 = K^T blocks
            QK = persist.tile([128, 12, T], bf, tag="QK")

            # W1 column slot for logical m-block under the host reorder
            # [m0, m6, m1, m7, m2, m8, m3, m9, m4, m10, m5, m11]
            QKSLOT = {}
            for i in range(6):
                QKSLOT[i] = 2 * i
                QKSLOT[6 + i] = 2 * i + 1

            def qk_mtile(m):
                # two 512-column chains, kc-interleaved so consecutive PE
                # writes alternate PSUM banks
                s = QKSLOT[m]
                psA = acc.tile([128, 512], f32, tag="acc")
                psB = acc.tile([128, 512], f32, tag="acc")
                for kc in range(KC):
                    nc.tensor.matmul(
                        psA[:], W1bf[:, kc, s * 128:(s + 1) * 128],
                        XT[:, kc, 0:512], start=(kc == 0), stop=(kc == KC - 1))
                    nc.tensor.matmul(
                        psB[:], W1bf[:, kc, s * 128:(s + 1) * 128],
                        XT[:, kc, 512:1024], start=(kc == 0), stop=(kc == KC - 1))
                # evictions on DVE: ScalarE is the critical engine (exp) and
                # must not queue eviction work ahead of the exp stream
                if m < 6:
                    nc.vector.tensor_scalar_add(QK[:, m, 0:512], psA[:], b1qk[:, m:m + 1])
                    nc.vector.tensor_scalar_add(QK[:, m, 512:1024], psB[:], b1qk[:, m:m + 1])
                else:
                    nc.vector.tensor_copy(QK[:, m, 0:512], psA[:])
                    nc.vector.tensor_copy(QK[:, m, 512:1024], psB[:])

            # ---- V projection into Vg[t-part, kt, h, 0:64]; cols 64:128 hold a
            # ones block so the AV matmul lands the softmax denominator,
            # replicated across PSUM partitions 64:128, for free.
            Vg = persist.tile([128, MT, H, 128], bf, tag="Vg")
            for mt in range(MT):
                nc.gpsimd.memset(Vg[:, mt, :, Dh:128], 1.0)

            def vproj(mts):
                for mt in mts:
                    psA = acc.tile([128, 512], f32, tag="acc")
                    psB = acc.tile([128, 512], f32, tag="acc")
                    for kc in range(KC):
                        nc.tensor.matmul(
                            psA[:, 0:512], XT[:, kc, mt * 128:(mt + 1) * 128],
                            W1bf[:, kc, 1536:2048],
                            start=(kc == 0), stop=(kc == KC - 1))
                        nc.tensor.matmul(
                            psB[:, 0:256], XT[:, kc, mt * 128:(mt + 1) * 128],
                            W1bf[:, kc, 2048:2304],
                            start=(kc == 0), stop=(kc == KC - 1))
                    nc.vector.tensor_add(
                        Vg[:, mt, 0:8, 0:Dh],
                        psA[:].rearrange("p (h d) -> p h d", d=Dh),
                        b1vb[:, 0:512].rearrange("p (h d) -> p h d", d=Dh))
                    nc.vector.tensor_add(
                        Vg[:, mt, 8:12, 0:Dh],
                        psB[:, 0:256].rearrange("p (h d) -> p h d", d=Dh),
                        b1vb[:, 512:768].rearrange("p (h d) -> p h d", d=Dh))

            # ---- attention ----
            ZT = persist.tile([128, NPAIR, T], bf, tag="ZT")

            def scores_kt(hp, kt, pts):
                L = T - kt * 128
                ptile = ptpool.tile([128, 2, 1024], bf, tag="pt", bufs=10)
                for c_off in range(0, L, 512):
                    n = min(512, L - c_off)
                    sc = att.tile([128, 2, 512], f32, tag="att")
                    for h01 in range(2):
                        base = h01 * 64
                        nc.tensor.matmul(
                            sc[:, h01, 0:n],
                            QK[base:base + 64, 6 + hp, kt * 128:(kt + 1) * 128],
                            QK[base:base + 64, hp, kt * 128 + c_off:kt * 128 + c_off + n],
                            start=True, stop=True)
                    nc.scalar.activation(
                        ptile[:, :, c_off:c_off + n], sc[:, :, 0:n], EXP, scale=SCALE)
                # causal mask on the diagonal 128x128 block, both heads in one op
                nc.vector.tensor_mul(ptile[:, :, 0:128], ptile[:, :, 0:128], diagmask[:])
                pts[kt] = (ptile, 0)

            def scores_kt67(hp, pts):
                # kt=6 (256 cols) and kt=7 (128 cols) share one PSUM tile
                # ([128,2,384] f32 = 3KB/part, fits the 2-bank att slot) and a
                # single exp - one ScalarE instruction instead of two.
                ptile = ptpool.tile([128, 2, 384], bf, tag="pt67", bufs=3)
                # full [2,512] slot so h01=1 starts on the PSUM bank boundary
                # (matmul outputs must not straddle banks); only 0:384 used
                sc = att.tile([128, 2, 512], f32, tag="att")
                for j, kt in enumerate((6, 7)):
                    off = 0 if j == 0 else 256
                    L = T - kt * 128
                    for h01 in range(2):
                        base = h01 * 64
                        nc.tensor.matmul(
                            sc[:, h01, off:off + L],
                            QK[base:base + 64, 6 + hp, kt * 128:(kt + 1) * 128],
                            QK[base:base + 64, hp, kt * 128:T],
                            start=True, stop=True, skip_group_check=True)
                nc.scalar.activation(ptile[:], sc[:, :, 0:384], EXP, scale=SCALE)
                nc.vector.tensor_mul(ptile[:, :, 0:128], ptile[:, :, 0:128], diagmask[:])
                nc.vector.tensor_mul(ptile[:, :, 256:384], ptile[:, :, 256:384], diagmask[:])
                pts[6] = (ptile, 0)
                pts[7] = (ptile, 256)

            def av_mm(hp, c, pts, keep_z=False):
                z = att.tile([128, 2, 512], f32, tag="att")
                kts = list(range(0, min(MT, 4 * (c + 1))))
                for kt in kts:
                    zoff = max(kt * 128 - c * 512, 0)
                    n = 512 - zoff
                    poff = max(c * 512 - kt * 128, 0)
                    ptile, pbase = pts[kt]
                    for h01 in range(2):
                        nc.tensor.matmul(
                            z[:, h01, zoff:zoff + n],
                            Vg[:, kt, 2 * hp + h01, :],
                            ptile[:, h01, pbase + poff:pbase + poff + n],
                            start=(kt == kts[0]), stop=(kt == kts[-1]),
                            skip_group_check=True)
                if keep_z:
                    # last pair: normalize straight from PSUM (saves the copy
                    # latency on the critical tail); z slot held until the muls.
                    return z
                # single bf16 copy (Z rows + denominator rows) frees the PSUM
                # slot; normalization is emitted later (av_norm) so ScalarE
                # keeps prioritizing the exp stream.
                zsb = zsbpool.tile([128, 2, 512], bf, tag="zsb")
                nc.vector.tensor_copy(zsb[:], z[:])
                return zsb

            def av_norm(hp, c, zsb, halves=1):
                # reciprocal of the denominator via Ln -> Exp(-x); the Ln reads
                # the denominator rows (partitions 64:128) and writes partitions
                # 0:64 (partition-crossed ACT), so the muls are lane-aligned.
                # halves=2 pipelines the chain in 256-col pieces for the
                # latency-critical last pair. zsb may be the live PSUM z tile
                # (keep_z path): the muls then read Z straight from PSUM.
                w = 512 // halves
                for hf in range(halves):
                    lo, hi = hf * w, (hf + 1) * w
                    dln = dlnpool.tile([64, 2, 512], f32, tag="dln")
                    nc.scalar.activation(dln[:, :, 0:w], zsb[Dh:128, :, lo:hi], LN)
                    rec = recpool.tile([64, 2, 512], bf, tag="rec")
                    nc.scalar.activation(rec[:, :, 0:w], dln[:, :, 0:w], EXP, scale=-1.0)
                    for h01 in range(2):
                        nc.vector.tensor_mul(
                            ZT[h01 * 64:(h01 + 1) * 64, hp, c * 512 + lo:c * 512 + hi],
                            zsb[0:Dh, h01, lo:hi], rec[:, h01, 0:w])

            # ---- output projection, split-K: pairs 0-3 accumulated early into
            # outA, pairs 4-5 at the tail; the merge is a ScalarE cast + DVE add
            # (no identity matmuls on the PE).
            outA = persist.tile([128, MT, E], bf, tag="outA")

            def outproj_a(mts):
                for mt in mts:
                    psA = acc.tile([128, 512], f32, tag="acc")
                    psB = acc.tile([128, 512], f32, tag="acc")
                    for pc in range(4):
                        nc.tensor.matmul(
                            psA[:], ZT[:, pc, mt * 128:(mt + 1) * 128],
                            W2bf[:, pc, 0:512], start=(pc == 0), stop=(pc == 3))
                        nc.tensor.matmul(
                            psB[:, 0:256], ZT[:, pc, mt * 128:(mt + 1) * 128],
                            W2bf[:, pc, 512:768], start=(pc == 0), stop=(pc == 3))
                    nc.vector.tensor_add(outA[:, mt, 0:512], psA[:], b2b[:, 0:512])
                    nc.vector.tensor_add(outA[:, mt, 512:768], psB[:, 0:256], b2b[:, 512:768])

            def outproj_b(mts):
                # pairs 4..5 tail on att-pool PSUM slots (3 in flight); the outA
                # partial (which already carries b2) is merged during eviction:
                # ScalarE casts the PSUM chain to bf16, DVE adds outA, one DMA.
                for mt in mts:
                    ps = att.tile([128, E], f32, tag="att")
                    for pc in range(4, KC):
                        nc.tensor.matmul(
                            ps[:, 0:512], ZT[:, pc, mt * 128:(mt + 1) * 128],
                            W2bf[:, pc, 0:512], start=(pc == 4), stop=(pc == KC - 1))
                        nc.tensor.matmul(
                            ps[:, 512:768], ZT[:, pc, mt * 128:(mt + 1) * 128],
                            W2bf[:, pc, 512:768], start=(pc == 4), stop=(pc == KC - 1),
                            skip_group_check=True)
                    osb = opool.tile([128, E], bf, tag="osb")
                    if mt == MT - 1:
                        # last tile: one-hop mixed-dtype add, shorter final drain
                        nc.vector.tensor_add(osb[:], ps[:], outA[:, mt, :])
                    else:
                        tmpb = opool.tile([128, E], bf, tag="tmpb")
                        nc.scalar.activation(tmpb[:], ps[:], COPY)
                        nc.vector.tensor_add(osb[:], tmpb[:], outA[:, mt, :])
                    nc.sync.dma_start(
                        out=out_ext[mt * 128:(mt + 1) * 128, :], in_=osb[:])

            # Software-pipelined pair loop: pair hp+1's first score batch is
            # emitted inside pair hp's body (before av_norm(hp,1)) so the next
            # exps queue on ScalarE ahead of the non-critical normalization
            # work instead of behind it. qk tiles are computed two pairs ahead.
            qk_mtile(0)
            qk_mtile(6)

            pts_cur = {}
            for kt in range(4):
                scores_kt(0, kt, pts_cur)
            qk_mtile(1)
            qk_mtile(7)

            for hp in range(NPAIR):
                if hp == 0:
                    vproj(range(MT))
                zsb0 = av_mm(hp, 0, pts_cur)
                if hp == NPAIR - 1:
                    # last pair: norm(5,0) jumps the ScalarE queue ahead of the
                    # kt4-7 exps - it only needs ptiles 0-3, and outproj_b(0..3)
                    # is gated on its ZT writes.
                    av_norm(hp, 0, zsb0, halves=2)
                scores_kt(hp, 4, pts_cur)
                scores_kt(hp, 5, pts_cur)
                scores_kt67(hp, pts_cur)
                if hp != NPAIR - 1:
                    av_norm(hp, 0, zsb0)
                if hp + 2 < NPAIR:
                    qk_mtile(hp + 2)
                    qk_mtile(6 + hp + 2)
                elif hp == 4:
                    # pair 4 has no qk tiles left to fill this slot; pairs 0-3
                    # are complete, so the first half of outproj_a covers the
                    # av_mm(4,1)-vs-exp race here instead of running after
                    # av_norm(4,1).
                    outproj_a(range(0, 4))
                zsb1 = av_mm(hp, 1, pts_cur)
                if hp == 1:
                    w2_dma()
                pts_next = {}
                if hp + 1 < NPAIR:
                    for kt in range(4):
                        scores_kt(hp + 1, kt, pts_next)
                av_norm(hp, 1, zsb1, halves=(2 if hp == NPAIR - 1 else 1))
                pts_cur = pts_next

            outproj_a(range(4, MT))
            outproj_b(range(0, 4))
            outproj_b(range(4, MT))

    nc.compile()
    return nc


def _get_nc():
    global _NC_CACHE
    if _NC_CACHE is None:
        _NC_CACHE = build_nc()
    return _NC_CACHE


def _in_maps(X, W1, b1, W2, b2):
    bfdt = ml_dtypes.bfloat16
    X = np.asarray(X, dtype=np.float32)
    assert X.shape == (B, T, E)
    W1 = np.asarray(W1, dtype=np.float32)
    # reorder QK columns into m-block pairs [m0|m6|m1|m7|...|m5|m11], V last,
    # matching the device-side QKSLOT map and DMA wave order
    order = []
    for i in range(6):
        order += [i, 6 + i]
    W1r = np.concatenate(
        [W1[:, m * 128:(m + 1) * 128] for m in order] + [W1[:, 1536:2304]], axis=1)
    W1b = np.ascontiguousarray(W1r.astype(bfdt))
    W2b = np.ascontiguousarray(np.asarray(W2, dtype=np.float32).astype(bfdt))
    b1 = np.ascontiguousarray(np.asarray(b1, dtype=np.float32))
    b1q = np.ascontiguousarray(b1[0:768].reshape(6, 128).T)
    b2 = np.ascontiguousarray(np.asarray(b2, dtype=np.float32))
    XTs = [np.ascontiguousarray(X[i].T.astype(bfdt)) for i in range(B)]
    return [
        {"XT": XTs[i], "W1": W1b, "b1": b1, "b1q": b1q, "W2": W2b, "b2": b2}
        for i in range(B)
    ]


def kernel(X, W1, b1, W2, b2):
    from concourse.bass_utils import run_bass_kernel_spmd

    nc = _get_nc()
    res = run_bass_kernel_spmd(nc, _in_maps(X, W1, b1, W2, b2), core_ids=list(range(B)))
    return np.stack([np.asarray(res.results[i]["out"], dtype=np.float32) for i in range(B)])


def kernel_traced(X, W1, b1, W2, b2, tmpdir=None):
    """Like kernel() but with neuron-profile tracing; returns (out, BassKernelResults)."""
    from concourse.bass_utils import run_bass_kernel_spmd

    nc = _get_nc()
    res = run_bass_kernel_spmd(
        nc, _in_maps(X, W1, b1, W2, b2), core_ids=list(range(B)),
        trace=True, tmpdir=tmpdir,
    )
    out = np.stack([np.asarray(res.results[i]["out"], dtype=np.float32) for i in range(B)])
    return out, res
